# revision 1
# baseline (speedup 1.0000x reference)
"""FEELVOS fused kernel for TRN2, 8-core SPMD.

Sharding: the reference only returns logits for classes C-2, C-1, so only 4 of
the 8 fused (batch, class) items matter. 8 cores = 4 (b, c) pairs x 2 frame
halves (top/bottom 24 rows). Bottom-half cores receive row-flipped inputs and
row-flipped conv kernels so every core runs the identical program computing
"top 25 rows" of its (possibly flipped) frame; the host un-flips on gather.

Per core:
  - partial U-Net on x3[b] (top-aligned row windows; convs as im2col or
    9-shifted matmuls on zero-padded [C, 50x50] SBUF planes, fp32r)
  - 100-d embeddings of x1[b,c], x2[b,c] (full frame, ref side) and of the
    class-c channel of the U-Net output (25-row window, query side)
  - distance matrix via one K=101 matmul per tile: rows 0..99 carry
    e3 . (-2 e_ref), row 100 carries 1 * |a_p|^2; DVE reduce_min over the
    free axis; |b_q|^2 added after the min (per-chunk column matmul);
    then tanh(d/2) == 1 - 2*sigmoid(-d)
  - 3x3 head conv (im2col, K=63) over [x3s(4ch), gm, lm, x2] -> 24 rows.
"""
import numpy as np

import concourse.bass as bass
import concourse.bacc as bacc
import concourse.tile as tile
from concourse import mybir
from concourse.bass_utils import run_bass_kernel_spmd
from concourse.masks import make_identity
from concourse.tile import add_dep_helper

F32 = mybir.dt.float32
F32R = mybir.dt.float32r
AF = mybir.ActivationFunctionType
ALU = mybir.AluOpType
AX = mybir.AxisListType

H = W = 48
NPIX = H * W                 # 2304 ref pixels
QROWS = 25
Q = QROWS * W                # 1200 query pixels
QCH, NQC = 120, 10           # query chunking for the distance matmul
PCH = [(0, 512), (512, 512), (1024, 512), (1536, 512), (2048, 256)]

_PROG = None


def _r3(ap, h, w):
    return ap.rearrange("c (h w) -> c h w", h=h, w=w)



# blobW column layout: (name, rows, cols); all segments base partition 0.
WSEGS = [("enc1s", 27, 16), ("enc2", 16, 288), ("bott", 32, 576),
         ("dec2a", 64, 288), ("dec2b", 32, 288), ("dec1a", 32, 144),
         ("dec1b", 16, 144), ("out", 16, 4), ("emb", 10, 100),
         ("dsh", 63, 1), ("outc", 16, 1),
         ("b_enc1", 16, 1), ("b_enc2", 32, 1), ("b_bott", 64, 1),
         ("b_dec2", 32, 1), ("b_dec1", 16, 1), ("b_out", 4, 1),
         ("b_dsh", 1, 1), ("b_outc", 1, 1)]
WOFF = {}
_o = 0
for _nm, _r, _c in WSEGS:
    WOFF[_nm] = _o
    _o += _c
WCOLS = _o


def _emit(nc, tc, ctx):
    # ------------------------------------------------------------- dram io
    bW = nc.dram_tensor("blobW", [64, WCOLS], F32, kind="ExternalInput").ap()
    bX1 = nc.dram_tensor("blobX1", [27, 39 * W], F32,
                         kind="ExternalInput").ap()
    bX2 = nc.dram_tensor("blobX2", [10, 2 * NPIX], F32,
                         kind="ExternalInput").ap()
    bX3 = nc.dram_tensor("blobX3", [1, Q], F32, kind="ExternalInput").ap()
    out_d = nc.dram_tensor("out", [24, W], F32, kind="ExternalOutput").ap()

    # round-robin DMA dispatch over the two HWDGE engines
    _dmaq = [nc.sync, nc.scalar]
    _qi = [0]

    def dma(out, in_):
        eng = _dmaq[_qi[0] % len(_dmaq)]
        _qi[0] += 1
        return eng.dma_start(out, in_)

    # ------------------------------------------------------------- sbuf
    sb = ctx.enter_context(tc.tile_pool(name="sb", bufs=1))

    def st(name, p, f, dt=F32):
        return sb.tile([p, f], dt, tag=name, name=name)

    blobw = st("blobw", 64, WCOLS, F32R)
    im27 = st("im27", 27, 39 * W, F32R)     # enc1 im2col (host-built)
    im2c12 = st("im2c12", 10, 2 * NPIX, F32R)  # emb im2col e1|e2 (host-built)
    xt6row = st("xt6row", 1, Q, F32R)

    def wseg(nm):
        rows, cols = next((r, c) for n, r, c in WSEGS if n == nm)
        return blobw[0:rows, WOFF[nm]:WOFF[nm] + cols]

    wt = {nm: wseg(nm) for nm, _, _ in WSEGS if not nm.startswith("b_")}
    bia = {nm[2:]: wseg(nm).bitcast(F32)
           for nm, _, _ in WSEGS if nm.startswith("b_")}

    # device-written padded planes
    x3cp = st("x3cp", 1, 2500, F32R)
    e1p = st("e1p", 16, 2500, F32R)
    p1p = st("p1p", 16, 676, F32R)
    e2p = st("e2p", 32, 676, F32R)
    p2p = st("p2p", 32, 196, F32R)
    btp = st("btp", 64, 196, F32R)
    ubp = st("ubp", 64, 676, F32R)
    d2p = st("d2p", 32, 676, F32R)
    udp = st("udp", 32, 2500, F32R)
    d1p = st("d1p", 16, 2500, F32R)
    xt = st("xt", 7, 2500, F32R)

    im2c3 = st("im2c3", 10, Q, F32R)        # emb im2col (e3)
    imdsh = st("imdsh", 63, 24 * W, F32R)   # head im2col (rows s*7+ci)
    e1x = st("e1x", 101, NPIX, F32R)
    e2x = st("e2x", 101, NPIX, F32R)
    e3x = st("e3x", 101, Q, F32R)
    esq = st("esq", 100, NPIX, F32R)
    ident = st("ident", 128, 128)

    c025 = st("c025", 100, 1, F32R)
    c1 = st("c1", 100, 2, F32R)
    a2row1 = st("a2row1", 1, NPIX, F32R)
    a2row2 = st("a2row2", 1, NPIX, F32R)
    gflat = st("gflat", 1, Q, F32R)
    lflat = st("lflat", 1, Q, F32R)
    out_sb = st("out_sb", 1, 24 * W)

    small = ctx.enter_context(tc.tile_pool(name="small", bufs=8))
    tmp = ctx.enter_context(tc.tile_pool(name="tmp", bufs=2))

    # ------------------------------------------------------------- init
    dma(im27[:], bX1.bitcast(F32R))
    dma(blobw[:], bW.bitcast(F32R))
    dma(im2c12[:], bX2.bitcast(F32R))
    dma(xt6row[:], bX3.bitcast(F32R))
    make_identity(nc, ident[:])

    xt3 = _r3(xt[:], 50, 50)
    x3cp3 = _r3(x3cp[:], 50, 50)

    # borders of device-written planes; xt fully (ch 4/5 are read by the
    # early imdsh build before gm/lm land, then patched)
    def borders(t, pw):
        v = _r3(t[:], pw, pw).bitcast(F32)
        return [nc.gpsimd.memset(v[:, 0:1, :], 0.0),
                nc.gpsimd.memset(v[:, pw - 1:pw, :], 0.0),
                nc.gpsimd.memset(v[:, 1:pw - 1, 0:1], 0.0),
                nc.gpsimd.memset(v[:, 1:pw - 1, pw - 1:pw], 0.0)]

    h_xt_ms = nc.gpsimd.memset(xt[:].bitcast(F32), 0.0)
    h_x3cp_b = borders(x3cp, 50)
    for t, pw in ((e1p, 50), (p1p, 26), (e2p, 26), (p2p, 14),
                  (btp, 14), (ubp, 26), (d2p, 26), (udp, 50)):
        borders(t, pw)
    nc.vector.memset(e3x[96:101, :].bitcast(F32), 1.0)   # row 100 = ones
    nc.vector.memset(im2c3[:].bitcast(F32), 1.0)
    nc.gpsimd.memset(c025[:].bitcast(F32), 0.25)
    nc.gpsimd.memset(c1[:].bitcast(F32), 1.0)
    # xt channel 6 = x2 rows 0..24
    h_xt6 = dma(xt3[6:7, 1:26, 1:49], xt6row[:])

    pconv = ctx.enter_context(tc.tile_pool(name="pconv", bufs=2, space="PSUM"))
    pmain = ctx.enter_context(tc.tile_pool(name="pmain", bufs=2, space="PSUM"))

    # ------------------------------------------------------------ helpers
    def conv9(srcs, cout, row_chunks, w_, func, bias_ap, dst3, scale=1.0):
        """3x3 conv via 9 shifted matmuls accumulating in PSUM.
        srcs: list of (plane3d, wtile, cin)."""
        r0 = 0
        for nr in row_chunks:
            ps = pconv.tile([cout, nr * w_], F32, tag="conv", name="convps")
            ops = []
            for (src3, wtile, cin) in srcs:
                for s in range(9):
                    dy, dx = s // 3, s % 3
                    ops.append((wtile[0:cin, s * cout:(s + 1) * cout],
                                src3[:, r0 + dy:r0 + dy + nr, dx:dx + w_]))
            for i, (l, r) in enumerate(ops):
                nc.tensor.matmul(ps[:], l, r.bitcast(F32R),
                                 start=(i == 0), stop=(i == len(ops) - 1))
            nc.scalar.activation(dst3[:, 1 + r0:1 + r0 + nr, 1:1 + w_],
                                 _r3(ps[:], nr, w_), func,
                                 bias=bias_ap, scale=scale)
            r0 += nr

    def shift_build(dst, plane3, ci, row0, rstep, nrows):
        """9 shift-DMAs: dst[row0 + s*rstep] = channel-ci window (dy, dx)."""
        for s in range(9):
            dy, dx = s // 3, s % 3
            dma(dst[row0 + s * rstep:row0 + s * rstep + 1, 0:nrows * W],
                plane3[ci:ci + 1, dy:dy + nrows, dx:dx + W])

    def conv_im2col(imbufs, cout, row_chunks, w_, func, bias_ap, dst3):
        r0 = 0
        for nr in row_chunks:
            ps = pconv.tile([cout, nr * w_], F32, tag="conv", name="convps")
            for i, (im, lhsT) in enumerate(imbufs):
                nc.tensor.matmul(ps[:], lhsT, im[:, r0 * w_:(r0 + nr) * w_],
                                 start=(i == 0), stop=(i == len(imbufs) - 1))
            nc.scalar.activation(dst3[:, 1 + r0:1 + r0 + nr, 1:1 + w_],
                                 _r3(ps[:], nr, w_), func, bias=bias_ap)
            r0 += nr

    def pool2(src3, dst3, orows, ocols, cch):
        t1 = tmp.tile([cch, orows * ocols], F32R, tag="pool_a", name="poolt1")
        t2 = tmp.tile([cch, orows * ocols], F32R, tag="pool_b", name="poolt2")
        v = [src3[:, 1 + a:1 + a + 2 * orows:2, 1 + b:1 + b + 2 * ocols:2]
             for a, b in ((0, 0), (1, 1), (0, 1), (1, 0))]
        nc.vector.tensor_max(_r3(t1[:], orows, ocols), v[0], v[1])
        nc.vector.tensor_max(_r3(t2[:], orows, ocols), v[2], v[3])
        nc.vector.tensor_max(dst3[:, 1:1 + orows, 1:1 + ocols],
                             _r3(t1[:], orows, ocols), _r3(t2[:], orows, ocols))

    def up2(src3, dst3, irows, icols):
        s = src3[:, 1:1 + irows, 1:1 + icols]
        for a in (0, 1):
            for b in (0, 1):
                nc.vector.tensor_copy(
                    dst3[:, 1 + a:1 + a + 2 * irows:2,
                         1 + b:1 + b + 2 * icols:2], s)

    def embconv(plane3, rows, imbuf, dst, scale):
        """1->100 3x3 conv via K=10 im2col matmul (row 9 = ones, wt row 9 =
        emb bias). plane3=None -> imbuf is prebuilt."""
        n = rows * W
        if plane3 is not None:
            for s in range(9):
                dy, dx = s // 3, s % 3
                dma(imbuf[s:s + 1, 0:n], plane3[0:1, dy:dy + rows, dx:dx + W])
        nch = 6 if rows == H else 3
        cw = n // nch
        for ci in range(nch):
            ps = pconv.tile([100, cw], F32, tag="conv", name="convps")
            nc.tensor.matmul(ps[:], wt["emb"],
                             imbuf[:, ci * cw:(ci + 1) * cw],
                             start=True, stop=True)
            nc.scalar.activation(dst[0:100, ci * cw:(ci + 1) * cw], ps[:],
                                 AF.Copy, scale=scale)

    def sqrow(ex, rowbuf):
        nc.scalar.activation(esq[:, 0:NPIX], ex[0:100, 0:NPIX], AF.Square)
        cw = NPIX // 6
        for ci in range(6):
            ps = pconv.tile([1, cw], F32, tag="conv", name="sqps")
            nc.tensor.matmul(ps[:], c025[:],
                             esq[:, ci * cw:(ci + 1) * cw],
                             start=True, stop=True)
            nc.scalar.copy(rowbuf[0:1, ci * cw:(ci + 1) * cw], ps[:])
        dma(ex[100:101, 0:NPIX], rowbuf[0:1, 0:NPIX])

    # ------------------------------------------------------------- U-Net
    e1p3 = _r3(e1p[:], 50, 50)
    p1p3 = _r3(p1p[:], 26, 26)
    e2p3 = _r3(e2p[:], 26, 26)
    p2p3 = _r3(p2p[:], 14, 14)
    btp3 = _r3(btp[:], 14, 14)
    ubp3 = _r3(ubp[:], 26, 26)
    d2p3 = _r3(d2p[:], 26, 26)
    udp3 = _r3(udp[:], 50, 50)
    d1p3 = _r3(d1p[:], 50, 50)

    conv_im2col([(im27[:], wt["enc1s"])], 16, [10, 10, 10, 8], W,
                AF.Relu, bia["enc1"], e1p3)
    pool2(e1p3, p1p3, 19, 24, 16)
    conv9([(p1p3, wt["enc2"], 16)], 32, [18], 24, AF.Relu, bia["enc2"], e2p3)
    pool2(e2p3, p2p3, 9, 12, 32)
    conv9([(p2p3, wt["bott"], 32)], 64, [8], 12, AF.Relu, bia["bott"], btp3)
    up2(btp3, ubp3, 8, 12)
    conv9([(ubp3, wt["dec2a"], 64), (e2p3, wt["dec2b"], 32)], 32, [14], 24,
          AF.Relu, bia["dec2"], d2p3)
    up2(d2p3, udp3, 14, 24)
    conv9([(udp3, wt["dec1a"], 32), (e1p3, wt["dec1b"], 16)], 16, [10, 10, 6],
          W, AF.Relu, bia["dec1"], d1p3)

    # 1x1 output conv -> xt[0:4] (all 4 channels) and x3cp (class-c channel)
    r0 = 0
    h_xt_ep, h_x3cp_ep = [], []
    for nr in (10, 10, 6):
        rhs = d1p3[:, 1 + r0:1 + r0 + nr, 1:1 + W]
        ps = pconv.tile([4, nr * W], F32, tag="conv", name="convps")
        nc.tensor.matmul(ps[:], wt["out"], rhs.bitcast(F32R),
                         start=True, stop=True)
        h_xt_ep.append(nc.scalar.activation(
            xt3[0:4, 1 + r0:1 + r0 + nr, 1:1 + W],
            _r3(ps[:], nr, W), AF.Identity, bias=bia["out"]))
        psc = pconv.tile([1, nr * W], F32, tag="conv", name="convps")
        nc.tensor.matmul(psc[:], wt["outc"], rhs.bitcast(F32R),
                         start=True, stop=True)
        h_x3cp_ep.append(nc.scalar.activation(
            x3cp3[0:1, 1 + r0:1 + r0 + nr, 1:1 + W],
            _r3(psc[:], nr, W), AF.Identity, bias=bia["outc"]))
        r0 += nr

    # early head-conv im2col, rows s*7+ci (gm/lm rows hold zeros from the
    # xt memset; patched after the matching).
    for s in range(9):
        dy, dx = s // 3, s % 3
        dma(imdsh[s * 7:(s + 1) * 7, 0:24 * W],
            xt3[:, dy:dy + 24, dx:dx + W])

    # ------------------------------------------------- embeddings (filler)
    embconv(None, H, im2c12[0:10, 0:NPIX], e1x[:], -2.0)
    sqrow(e1x[:], a2row1[:])
    embconv(None, H, im2c12[0:10, NPIX:2 * NPIX], e2x[:], -2.0)
    sqrow(e2x[:], a2row2[:])

    # ------------------------------------------------------- embedding 3
    shift_build(im2c3[:], x3cp3, 0, 0, 1, QROWS)
    embconv(None, QROWS, im2c3[:], e3x[:], 1.0)
    nc.scalar.activation(esq[:, 0:Q], e3x[0:100, 0:Q], AF.Square)

    # ------------------------------------------------------- matching
    # ref-major: all gm chunks first so the gm plane + its imdsh patch DMAs
    # hide behind lm's compute. |b|^2 columns computed once, cached.
    b2cs = []
    for c in range(NQC):
        b2ps = pconv.tile([QCH, 2], F32, tag="conv", name="b2ps")
        nc.tensor.matmul(b2ps[:], esq[:, c * QCH:(c + 1) * QCH], c1[:],
                         start=True, stop=True)
        b2c = small.tile([QCH, 1], F32, tag="b2c", name="b2c", bufs=10)
        nc.scalar.copy(b2c[:], b2ps[:, 0:1])
        b2cs.append(b2c)
    for r, ex in enumerate((e1x, e2x)):
        flat = gflat if r == 0 else lflat
        for c in range(NQC):
            lhsT = e3x[:, c * QCH:(c + 1) * QCH]
            mins = small.tile([QCH, 3], F32, tag="mins", name="mins")
            for j in range(2):
                ps = pmain.tile([QCH, 1024], F32, tag="main", name="mainps")
                nc.tensor.matmul(ps[:, 0:512], lhsT,
                                 ex[:][:, j * 1024:j * 1024 + 512],
                                 start=True, stop=True)
                nc.tensor.matmul(ps[:, 512:1024], lhsT,
                                 ex[:][:, j * 1024 + 512:(j + 1) * 1024],
                                 start=True, stop=True)
                nc.vector.tensor_reduce(mins[:, j:j + 1], ps[:],
                                        axis=AX.X, op=ALU.min)
            pt = pmain.tile([QCH, 256], F32, tag="maint", name="maintps",
                            bufs=2)
            nc.tensor.matmul(pt[:], lhsT, ex[:][:, 2048:2304],
                             start=True, stop=True)
            nc.vector.tensor_reduce(mins[:, 2:3], pt[:], axis=AX.X,
                                    op=ALU.min)
            dmin = small.tile([QCH, 1], F32, tag="dmin", name="dmin")
            nc.vector.tensor_reduce(dmin[:], mins[:], axis=AX.X, op=ALU.min)
            dmax = small.tile([QCH, 1], F32, tag="dmax", name="dmax")
            nc.vector.tensor_scalar(dmax[:], dmin[:], b2cs[c][:], 0.0,
                                    op0=ALU.add, op1=ALU.max)
            gcol = small.tile([QCH, 1], F32, tag="gcol", name="gcol")
            nc.scalar.activation(gcol[:], dmax[:], AF.Tanh, scale=0.5)
            pst = pconv.tile([1, QCH], F32, tag="conv", name="gmtps")
            nc.tensor.transpose(pst[:], gcol[:], ident[:QCH, :QCH])
            nc.scalar.copy(flat[0:1, c * QCH:(c + 1) * QCH], pst[:])
        # plane write + imdsh patch for this ref
        plane_h = dma(xt3[4 + r:5 + r, 1:26, 1:49],
                      _r3(flat[:], QROWS, W)[:, :, :])
        shift_build(imdsh[:], xt3, 4 + r, 4 + r, 7, 24)

    r0 = 0
    for nr in (8, 8, 8):
        ps = pconv.tile([1, nr * W], F32, tag="conv", name="convps")
        nc.tensor.matmul(ps[:], wt["dsh"],
                         imdsh[:, r0 * W:(r0 + nr) * W],
                         start=True, stop=True)
        nc.scalar.activation(out_sb[0:1, r0 * W:(r0 + nr) * W],
                             _r3(ps[:], nr, W), AF.Identity,
                             bias=bia["dsh"])
        r0 += nr
    nc.sync.dma_start(out_d, out_sb[:])


def build_program():
    import contextlib
    nc = bacc.Bacc("TRN2", target_bir_lowering=False, debug=False,
                   num_devices=8)
    with tile.TileContext(nc) as tc:
        with contextlib.ExitStack() as ctx:
            _emit(nc, tc, ctx)
    nc.compile()
    return nc


def _get_program():
    global _PROG
    if _PROG is None:
        _PROG = build_program()
    return _PROG


CORE_BC = [(0, 2), (0, 3), (1, 2), (1, 3)]


def _wT_flat(w):
    """[Cout, Cin, 3, 3] -> [Cin, 9*Cout]: col block s holds w[:, :, s//3, s%3].T"""
    cout, cin = w.shape[:2]
    out = np.zeros((cin, 9 * cout), np.float32)
    for s in range(9):
        out[:, s * cout:(s + 1) * cout] = w[:, :, s // 3, s % 3].T
    return out


def _pad50(img):
    out = np.zeros((50, 50), np.float32)
    out[1:49, 1:49] = img
    return out


def _im2col9(img, rows, ones_row=False):
    """padded 50x50 -> [9(+1), rows*48] rows ordered s=dy*3+dx."""
    p = _pad50(img)
    rws = [p[dy:dy + rows, dx:dx + W].ravel()
           for dy in range(3) for dx in range(3)]
    if ones_row:
        rws.append(np.ones(rows * W, np.float32))
    return np.stack(rws)


def _blobw(inp, flip, c):
    w = {k: (inp[k][:, :, ::-1, :] if flip else inp[k])
         for k in ["enc1_w", "enc2_w", "bott_w", "dec2_w", "dec1_w",
                   "emb_w", "dsh_w"]}
    seg = {}
    seg["enc1s"] = w["enc1_w"].reshape(16, 3, 9).transpose(2, 1, 0) \
                              .reshape(27, 16)
    seg["enc2"] = _wT_flat(w["enc2_w"])
    seg["bott"] = _wT_flat(w["bott_w"])
    seg["dec2a"] = _wT_flat(w["dec2_w"][:, :64])
    seg["dec2b"] = _wT_flat(w["dec2_w"][:, 64:])
    seg["dec1a"] = _wT_flat(w["dec1_w"][:, :32])
    seg["dec1b"] = _wT_flat(w["dec1_w"][:, 32:])
    seg["out"] = inp["out_w"][:, :, 0, 0].T
    seg["emb"] = np.vstack([w["emb_w"].reshape(100, 9).T,
                            inp["emb_b"][None, :]])
    seg["dsh"] = w["dsh_w"].reshape(7, 9).T.reshape(63, 1)
    seg["outc"] = inp["out_w"][c, :, 0, 0][:, None]
    for k in ["enc1", "enc2", "bott", "dec2", "dec1", "out", "dsh"]:
        seg["b_" + k] = inp[k + "_b"][:, None]
    seg["b_outc"] = inp["out_b"][c:c + 1][:, None]
    blob = np.zeros((64, WCOLS), np.float32)
    for nm, rows, cols in WSEGS:
        blob[0:rows, WOFF[nm]:WOFF[nm] + cols] = seg[nm]
    return blob


def make_in_maps(inp):
    maps = []
    for k8 in range(8):
        n_idx, half = k8 // 2, k8 % 2
        b, c = CORE_BC[n_idx]
        x1c, x2c, x3b = inp["x1"][b, c], inp["x2"][b, c], inp["x3"][b]
        if half:
            x1c, x2c, x3b = x1c[::-1], x2c[::-1], x3b[:, ::-1]
        # enc1 im2col [27, 39*48]: row s*3+ci
        bx1 = np.zeros((27, 39 * W), np.float32)
        for ci in range(3):
            im9 = _im2col9(x3b[ci], 39)
            for s in range(9):
                bx1[s * 3 + ci] = im9[s]
        bx2 = np.concatenate([_im2col9(x1c, H, True),
                              _im2col9(x2c, H, True)], axis=1)
        bx3 = x2c[0:25, :].reshape(1, Q)
        maps.append({"blobW": np.ascontiguousarray(_blobw(inp, bool(half), c)),
                     "blobX1": np.ascontiguousarray(bx1),
                     "blobX2": np.ascontiguousarray(bx2),
                     "blobX3": np.ascontiguousarray(bx3)})
    return maps


def assemble(results):
    out = np.zeros((2, 2, H, W), np.float32)
    for k8, r in enumerate(results):
        n_idx, half = k8 // 2, k8 % 2
        b, c = CORE_BC[n_idx]
        y = r["out"]
        if half == 0:
            out[b, c - 2, 0:24] = y
        else:
            out[b, c - 2, 24:48] = y[::-1]
    return out


def kernel(**inputs):
    inp = {k: np.asarray(v) for k, v in inputs.items()}
    nc = _get_program()
    maps = make_in_maps(inp)
    res = run_bass_kernel_spmd(nc, maps, core_ids=list(range(8)), trace=False)
    return assemble(res.results)



# revision 9
# speedup vs baseline: 1.0309x; 1.0309x over previous
"""FEELVOS fused kernel for TRN2, 8-core SPMD — fp16 rev.

Sharding: the reference only returns logits for classes C-2, C-1, so only 4 of
the 8 fused (batch, class) items matter. 8 cores = 4 (b, c) pairs x 2 frame
halves (top/bottom 24 rows). Bottom-half cores receive row-flipped inputs and
row-flipped conv kernels so every core runs the identical program computing
"top 25 rows" of its (possibly flipped) frame; the host un-flips on gather.

All matmul operands are fp16 (PE streams 1 col/cycle vs ~3 for fp32-HIGH);
PSUM accumulation stays fp32. The |a|^2 row of the distance matmul is carried
as an fp16 hi+lo pair (rows 100/101, K=102) to avoid fp16 quantization of a
large magnitude. Host-side numpy sim of this exact scheme: rel err 1.2e-3
(gate 2e-2).

Per core:
  - partial U-Net on x3[b] (top-aligned row windows; convs as im2col or
    9-shifted matmuls on zero-padded fp16 planes)
  - 100-d embeddings of x1[b,c], x2[b,c] (ref side, host im2col) and of the
    class-c channel of the U-Net output (25-row window, query side)
  - distance: per (ref, query-chunk) two [120,1152] PSUM tiles (3 matmuls
    each, K=102 incl. |a|^2 hi/lo vs ones), folded by one
    tensor_tensor_reduce(min,min) per tile; |b|^2 added after the min;
    tanh(d/2) == 1 - 2*sigmoid(-d); single [120,10]->[10,120] PE transpose
    per ref writes the plane.
  - 3x3 head conv directly on the padded [7,2500] xt plane (9 shifted
    matmuls, K=7) — no im2col, no patch DMAs.
"""
import numpy as np

import concourse.bass as bass
import concourse.bacc as bacc
import concourse.tile as tile
from concourse import mybir
from concourse.bass_utils import run_bass_kernel_spmd
from concourse.masks import make_identity

F32 = mybir.dt.float32
F16 = mybir.dt.float16
AF = mybir.ActivationFunctionType
ALU = mybir.AluOpType
AX = mybir.AxisListType

H = W = 48
NPIX = H * W                 # 2304 ref pixels
QROWS = 25
Q = QROWS * W                # 1200 query pixels
QCH, NQC = 120, 10           # query chunking for the distance matmul
ECH = [512, 512, 512, 512, 256]   # 2304 column chunking (PSUM bank)
E3CH = [432, 384, 384]            # 1200 column chunking

_PROG = None


def _r3(ap, h, w):
    return ap.rearrange("c (h w) -> c h w", h=h, w=w)


# blobA (hot) / blobB (cold) column layouts: (name, rows, cols), fp16.
ASEGS = [("enc1s", 27, 16), ("enc2", 16, 288), ("bott", 32, 576)]
BSEGS = [("dec2a", 64, 288), ("dec2b", 32, 288), ("dec1a", 32, 144),
         ("dec1b", 16, 144), ("out", 16, 4), ("outc", 16, 1),
         ("emb", 10, 100), ("dshc", 7, 9)]


def _offsets(segs):
    off, o = {}, 0
    for nm, _r, c in segs:
        off[nm] = o
        o += c
    return off, o


AOFF, ACOLS = _offsets(ASEGS)
BOFF, BCOLS = _offsets(BSEGS)
# blobBias [64, 8] fp32 columns
BIAS_COL = {"enc1": 0, "enc2": 1, "bott": 2, "dec2": 3, "dec1": 4,
            "out": 5, "dsh": 6, "outc": 7}


def _emit(nc, tc, ctx):
    # ------------------------------------------------------------- dram io
    bA = nc.dram_tensor("blobA", [64, ACOLS], F16, kind="ExternalInput").ap()
    bB = nc.dram_tensor("blobB", [64, BCOLS], F16, kind="ExternalInput").ap()
    bBias = nc.dram_tensor("blobBias", [64, 8], F32, kind="ExternalInput").ap()
    bX1 = nc.dram_tensor("blobX1", [27, 39 * W], F16,
                         kind="ExternalInput").ap()
    bX2 = nc.dram_tensor("blobX2", [10, 2 * NPIX], F16,
                         kind="ExternalInput").ap()
    bX3 = nc.dram_tensor("blobX3", [1, Q], F16, kind="ExternalInput").ap()
    out_d = nc.dram_tensor("out", [1, 24 * W], F32, kind="ExternalOutput").ap()

    # ------------------------------------------------------------- sbuf
    sb = ctx.enter_context(tc.tile_pool(name="sb", bufs=1))

    def st(name, p, f, dt=F16):
        return sb.tile([p, f], dt, tag=name, name=name)

    bloba = st("bloba", 64, ACOLS)
    blobb = st("blobb", 64, BCOLS)
    blobbias = st("blobbias", 64, 8, F32)
    im27 = st("im27", 27, 39 * W)        # enc1 im2col (host-built)
    im2c12 = st("im2c12", 10, 2 * NPIX)  # emb im2col e1|e2 (host-built)
    xt6row = st("xt6row", 1, Q)

    def wseg(blob, off, segs, nm):
        rows, cols = next((r, c) for n, r, c in segs if n == nm)
        return blob[0:rows, off[nm]:off[nm] + cols]

    wt = {nm: wseg(bloba, AOFF, ASEGS, nm) for nm, _, _ in ASEGS}
    wt.update({nm: wseg(blobb, BOFF, BSEGS, nm) for nm, _, _ in BSEGS})
    bia = {nm: blobbias[0:r, c:c + 1]
           for nm, (r, c) in {"enc1": (16, 0), "enc2": (32, 1),
                              "bott": (64, 2), "dec2": (32, 3),
                              "dec1": (16, 4), "out": (4, 5),
                              "dsh": (1, 6), "outc": (1, 7)}.items()}

    # device-written padded planes (all fp16)
    x3cp = st("x3cp", 1, 2500)
    e1p = st("e1p", 16, 2500)
    p1p = st("p1p", 16, 676)
    e2p = st("e2p", 32, 676)
    p2p = st("p2p", 32, 196)
    btp = st("btp", 64, 196)
    ubp = st("ubp", 64, 676)
    d2p = st("d2p", 32, 676)
    udp = st("udp", 32, 2500)
    d1p = st("d1p", 16, 2500)
    xt = st("xt", 7, 2500)

    im2c3 = st("im2c3", 10, Q)           # emb im2col (e3)
    e1x = st("e1x", 102, NPIX)           # rows 0..99 = -2*e1, 100/101 |a|^2 hi/lo
    e2x = st("e2x", 102, NPIX)
    e3x = st("e3x", 102, Q)              # rows 0..99 = e3, 100/101 = ones
    esq = st("esq", 100, NPIX)           # squares scratch (e1/e2/e3 serially)
    ident = st("ident", 128, 128)

    c025 = st("c025", 100, 1)
    c1 = st("c1", 100, 2)
    hib = st("hib", 1, NPIX)
    lob = st("lob", 1, NPIX)
    b2all = st("b2all", QCH, NQC, F32)
    minsG = st("minsG", QCH, 2 * NQC, F32)
    minsL = st("minsL", QCH, 2 * NQC, F32)
    out_sb = st("out_sb", 1, 24 * W, F32)

    small = ctx.enter_context(tc.tile_pool(name="small", bufs=8))
    tmp = ctx.enter_context(tc.tile_pool(name="tmp", bufs=2))
    scr = ctx.enter_context(tc.tile_pool(name="scr", bufs=2))

    # ------------------------------------------------------------- input dma
    # sync queue, hot-first so enc1 can start early
    nc.sync.dma_start(bloba[:, 0:16], bA[:, 0:16])          # enc1s
    nc.sync.dma_start(blobbias[:], bBias)
    nc.sync.dma_start(im27[:, 0:960], bX1[:, 0:960])        # enc1 rows 0..19
    nc.sync.dma_start(im27[:, 960:39 * W], bX1[:, 960:39 * W])
    nc.sync.dma_start(bloba[:, 16:ACOLS], bA[:, 16:ACOLS])  # enc2+bott
    nc.sync.dma_start(blobb[:], bB)
    nc.sync.dma_start(im2c12[:, 0:NPIX], bX2[:, 0:NPIX])
    nc.sync.dma_start(im2c12[:, NPIX:2 * NPIX], bX2[:, NPIX:2 * NPIX])
    nc.sync.dma_start(xt6row[:], bX3)

    # ------------------------------------------------------------- init
    make_identity(nc, ident[:])
    nc.gpsimd.memset(c025[:], 0.25)
    nc.gpsimd.memset(c1[:], 1.0)
    nc.gpsimd.memset(xt[:], 0.0)
    # engine partition starts must be 32-aligned; rows 96..99 / 0..8 are
    # overwritten later by the embconv ACT / shift DMAs.
    nc.vector.memset(e3x[96:102, :], 1.0)
    nc.vector.memset(im2c3[0:10, :], 1.0)

    xt3 = _r3(xt[:], 50, 50)
    x3cp3 = _r3(x3cp[:], 50, 50)

    def borders(eng, t, pw):
        v = _r3(t[:], pw, pw)
        eng.memset(v[:, 0:1, :], 0.0)
        eng.memset(v[:, pw - 1:pw, :], 0.0)
        eng.memset(v[:, 1:pw - 1, 0:1], 0.0)
        eng.memset(v[:, 1:pw - 1, pw - 1:pw], 0.0)

    borders(nc.gpsimd, x3cp, 50)
    for eng, grp in ((nc.gpsimd, ((e1p, 50), (p1p, 26), (e2p, 26), (p2p, 14))),
                    (nc.vector, ((btp, 14), (ubp, 26), (d2p, 26), (udp, 50)))):
        for t, pw in grp:
            borders(eng, t, pw)
    # xt ch6 = x2 rows 0..24 (gpsimd queue: ordered after the xt memset)
    nc.gpsimd.dma_start(xt3[6:7, 1:26, 1:49], xt6row[:])

    pconv = ctx.enter_context(tc.tile_pool(name="pconv", bufs=2, space="PSUM"))
    pmain = ctx.enter_context(tc.tile_pool(name="pmain", bufs=2, space="PSUM"))

    # ------------------------------------------------------------ helpers
    def conv9(srcs, cout, row_chunks, w_, func, bias_ap, dst3):
        """3x3 conv via 9 shifted matmuls accumulating in PSUM.
        srcs: list of (plane3d, wtile, cin)."""
        r0 = 0
        for nr in row_chunks:
            ps = pconv.tile([cout, nr * w_], F32, tag="conv", name="convps")
            ops = []
            for (src3, wtile, cin) in srcs:
                for s in range(9):
                    dy, dx = s // 3, s % 3
                    ops.append((wtile[0:cin, s * cout:(s + 1) * cout],
                                src3[:, r0 + dy:r0 + dy + nr, dx:dx + w_]))
            for i, (l, r) in enumerate(ops):
                nc.tensor.matmul(ps[:], l, r,
                                 start=(i == 0), stop=(i == len(ops) - 1))
            nc.scalar.activation(dst3[:, 1 + r0:1 + r0 + nr, 1:1 + w_],
                                 _r3(ps[:], nr, w_), func, bias=bias_ap)
            r0 += nr

    def conv_im2col(imbufs, cout, row_chunks, w_, func, bias_ap, dst3):
        r0 = 0
        for nr in row_chunks:
            ps = pconv.tile([cout, nr * w_], F32, tag="conv", name="convps")
            for i, (im, lhsT) in enumerate(imbufs):
                nc.tensor.matmul(ps[:], lhsT, im[:, r0 * w_:(r0 + nr) * w_],
                                 start=(i == 0), stop=(i == len(imbufs) - 1))
            nc.scalar.activation(dst3[:, 1 + r0:1 + r0 + nr, 1:1 + w_],
                                 _r3(ps[:], nr, w_), func, bias=bias_ap)
            r0 += nr

    def pool2(src3, dst3, orows, ocols, cch):
        t1 = tmp.tile([cch, orows * ocols], F16, tag="pool_a", name="poolt1")
        t2 = tmp.tile([cch, orows * ocols], F16, tag="pool_b", name="poolt2")
        v = [src3[:, 1 + a:1 + a + 2 * orows:2, 1 + b:1 + b + 2 * ocols:2]
             for a, b in ((0, 0), (1, 1), (0, 1), (1, 0))]
        nc.vector.tensor_max(_r3(t1[:], orows, ocols), v[0], v[1])
        nc.vector.tensor_max(_r3(t2[:], orows, ocols), v[2], v[3])
        nc.vector.tensor_max(dst3[:, 1:1 + orows, 1:1 + ocols],
                             _r3(t1[:], orows, ocols), _r3(t2[:], orows, ocols))

    def up2(src3, dst3, irows, icols):
        s = src3[:, 1:1 + irows, 1:1 + icols]
        for a in (0, 1):
            for b in (0, 1):
                nc.vector.tensor_copy(
                    dst3[:, 1 + a:1 + a + 2 * irows:2,
                         1 + b:1 + b + 2 * icols:2], s)

    def embconv(imbuf, chunks, dst, scale):
        """1->100 3x3 conv via K=10 im2col matmul (row 9 = ones carrying the
        emb bias in wt row 9)."""
        off = 0
        for cw in chunks:
            ps = pconv.tile([100, cw], F32, tag="conv", name="convps")
            nc.tensor.matmul(ps[:], wt["emb"], imbuf[:, off:off + cw],
                             start=True, stop=True)
            nc.scalar.activation(dst[0:100, off:off + cw], ps[:],
                                 AF.Copy, scale=scale)
            off += cw

    def sqrow(ex):
        """|a|^2 = 0.25*sum((-2e)^2) -> fp16 hi/lo pair in ex rows 100/101."""
        nc.scalar.activation(esq[:, 0:NPIX], ex[0:100, 0:NPIX], AF.Square)
        off = 0
        for cw in ECH:
            ps = pconv.tile([1, cw], F32, tag="conv", name="sqps")
            nc.tensor.matmul(ps[:], c025[:], esq[:, off:off + cw],
                             start=True, stop=True)
            nc.scalar.copy(hib[0:1, off:off + cw], ps[:])
            nc.vector.scalar_tensor_tensor(lob[0:1, off:off + cw], ps[:], 1.0,
                                           hib[0:1, off:off + cw],
                                           op0=ALU.mult, op1=ALU.subtract)
            off += cw
        nc.sync.dma_start(ex[100:101, 0:NPIX], hib[0:1, 0:NPIX])
        nc.sync.dma_start(ex[101:102, 0:NPIX], lob[0:1, 0:NPIX])

    # ------------------------------------------------------------- U-Net
    e1p3 = _r3(e1p[:], 50, 50)
    p1p3 = _r3(p1p[:], 26, 26)
    e2p3 = _r3(e2p[:], 26, 26)
    p2p3 = _r3(p2p[:], 14, 14)
    btp3 = _r3(btp[:], 14, 14)
    ubp3 = _r3(ubp[:], 26, 26)
    d2p3 = _r3(d2p[:], 26, 26)
    udp3 = _r3(udp[:], 50, 50)
    d1p3 = _r3(d1p[:], 50, 50)

    conv_im2col([(im27[:], wt["enc1s"])], 16, [10, 10, 10, 8], W,
                AF.Relu, bia["enc1"], e1p3)
    pool2(e1p3, p1p3, 19, 24, 16)
    conv9([(p1p3, wt["enc2"], 16)], 32, [18], 24, AF.Relu, bia["enc2"], e2p3)
    pool2(e2p3, p2p3, 9, 12, 32)
    conv9([(p2p3, wt["bott"], 32)], 64, [8], 12, AF.Relu, bia["bott"], btp3)
    up2(btp3, ubp3, 8, 12)
    conv9([(ubp3, wt["dec2a"], 64), (e2p3, wt["dec2b"], 32)], 32, [14], 24,
          AF.Relu, bia["dec2"], d2p3)
    up2(d2p3, udp3, 14, 24)
    conv9([(udp3, wt["dec1a"], 32), (e1p3, wt["dec1b"], 16)], 16, [10, 10, 6],
          W, AF.Relu, bia["dec1"], d1p3)

    # 1x1 output conv -> xt[0:4] (all 4 channels) and x3cp (class-c channel)
    r0 = 0
    for nr in (10, 10, 6):
        rhs = d1p3[:, 1 + r0:1 + r0 + nr, 1:1 + W]
        ps = pconv.tile([4, nr * W], F32, tag="conv", name="convps")
        nc.tensor.matmul(ps[:], wt["out"], rhs, start=True, stop=True)
        nc.scalar.activation(xt3[0:4, 1 + r0:1 + r0 + nr, 1:1 + W],
                             _r3(ps[:], nr, W), AF.Identity, bias=bia["out"])
        psc = pconv.tile([1, nr * W], F32, tag="conv", name="convps")
        nc.tensor.matmul(psc[:], wt["outc"], rhs, start=True, stop=True)
        nc.scalar.activation(x3cp3[0:1, 1 + r0:1 + r0 + nr, 1:1 + W],
                             _r3(psc[:], nr, W), AF.Identity, bias=bia["outc"])
        r0 += nr

    # ------------------------------------------------- ref embeddings
    embconv(im2c12[0:10, 0:NPIX], ECH, e1x[:], -2.0)
    sqrow(e1x[:])
    embconv(im2c12[0:10, NPIX:2 * NPIX], ECH, e2x[:], -2.0)
    sqrow(e2x[:])

    # ------------------------------------------------------- embedding 3
    for s in range(9):
        dy, dx = s // 3, s % 3
        eng = nc.sync if s % 2 == 0 else nc.gpsimd
        eng.dma_start(im2c3[s:s + 1, 0:Q],
                      x3cp3[0:1, dy:dy + QROWS, dx:dx + W])
    embconv(im2c3[:], E3CH, e3x[:], 1.0)
    nc.scalar.activation(esq[:, 0:Q], e3x[0:100, 0:Q], AF.Square)

    # |b|^2 columns [120, 1] per chunk -> b2all
    for c in range(NQC):
        b2ps = pconv.tile([QCH, 2], F32, tag="conv", name="b2ps")
        nc.tensor.matmul(b2ps[:], esq[:, c * QCH:(c + 1) * QCH], c1[:],
                         start=True, stop=True)
        nc.scalar.copy(b2all[:, c:c + 1], b2ps[:, 0:1])

    # ------------------------------------------------------- matching
    def match_chunks(ex, minsR, c_range):
        for c in c_range:
            lhsT = e3x[:, c * QCH:(c + 1) * QCH]
            offload = c in (1, 3, 5, 7)   # ACT converts, DVE reduces fp16
            for j in range(2):
                ps = pmain.tile([QCH, 1152], F32, tag="main", name="mainps")
                base = j * 1152
                for o, n in ((0, 512), (512, 512), (1024, 128)):
                    nc.tensor.matmul(ps[:, o:o + n], lhsT,
                                     ex[:][:, base + o:base + o + n],
                                     start=True, stop=True)
                col = minsR[:, 2 * c + j:2 * c + j + 1]
                if offload:
                    sc = scr.tile([QCH, 1152], F16, tag="scr", name="scrt")
                    nc.scalar.copy(sc[:], ps[:])
                    nc.vector.tensor_reduce(col, sc[:], axis=AX.X, op=ALU.min)
                else:
                    nc.vector.tensor_reduce(col, ps[:], axis=AX.X, op=ALU.min)

    def ref_tail(minsR):
        """min over col pairs, +|b|^2, clamp, tanh -> gcol [120,10] fp16."""
        dmin10 = small.tile([QCH, NQC], F32, tag="dmin", name="dmin")
        nc.vector.tensor_reduce(
            dmin10[:], minsR[:].rearrange("p (c two) -> p c two", two=2),
            axis=AX.X, op=ALU.min)
        dsum = small.tile([QCH, NQC], F32, tag="dsum", name="dsum")
        nc.vector.scalar_tensor_tensor(dsum[:], dmin10[:], 1.0, b2all[:],
                                       op0=ALU.mult, op1=ALU.add)
        dmax = small.tile([QCH, NQC], F32, tag="dmax", name="dmax")
        nc.vector.tensor_scalar(dmax[:], dsum[:], 0.0, 0.0,
                                op0=ALU.max, op1=ALU.max)
        gcol = small.tile([QCH, NQC], F16, tag="gcol", name="gcol")
        nc.scalar.activation(gcol[:], dmax[:], AF.Tanh, scale=0.5)
        return gcol

    def ref_emit_plane(gcol, r, eng):
        pst = pconv.tile([NQC, QCH], F16, tag="conv", name="gmtps")
        nc.tensor.transpose(pst[:], gcol[:], ident[:QCH, :QCH])
        gcolT = small.tile([NQC, QCH], F16, tag="gcolT", name="gcolT")
        nc.scalar.copy(gcolT[:], pst[:])
        gflat = small.tile([1, Q], F16, tag="gflat", name="gflat")
        eng.dma_start(gflat[:], gcolT[:])
        eng.dma_start(xt3[4 + r:5 + r, 1:26, 1:49], gflat[:])

    match_chunks(e1x, minsG, range(NQC))          # gm matmuls + reduces
    gcol_g = ref_tail(minsG)                      # overlaps lm matmuls below
    match_chunks(e2x, minsL, range(3))
    ref_emit_plane(gcol_g, 0, nc.gpsimd)          # PE transpose: no stall here
    match_chunks(e2x, minsL, range(3, NQC))
    gcol_l = ref_tail(minsL)
    ref_emit_plane(gcol_l, 1, nc.sync)

    # ------------------------------------------------------- head conv
    r0 = 0
    for nr in (8, 8, 8):
        ps = pconv.tile([1, nr * W], F32, tag="conv", name="convps")
        for s in range(9):
            dy, dx = s // 3, s % 3
            nc.tensor.matmul(ps[:], wt["dshc"][:, s:s + 1],
                             xt3[:, r0 + dy:r0 + dy + nr, dx:dx + W],
                             start=(s == 0), stop=(s == 8))
        nc.scalar.activation(out_sb[0:1, r0 * W:(r0 + nr) * W],
                             _r3(ps[:], nr, W), AF.Identity, bias=bia["dsh"])
        r0 += nr
    nc.sync.dma_start(out_d, out_sb[:])


def build_program():
    import contextlib
    nc = bacc.Bacc("TRN2", target_bir_lowering=False, debug=False,
                   num_devices=8)
    with tile.TileContext(nc) as tc:
        with contextlib.ExitStack() as ctx:
            _emit(nc, tc, ctx)
    nc.compile()
    return nc


def _get_program():
    global _PROG
    if _PROG is None:
        _PROG = build_program()
    return _PROG


CORE_BC = [(0, 2), (0, 3), (1, 2), (1, 3)]


def _wT_flat(w):
    """[Cout, Cin, 3, 3] -> [Cin, 9*Cout]: col block s holds w[:, :, s//3, s%3].T"""
    cout, cin = w.shape[:2]
    out = np.zeros((cin, 9 * cout), np.float32)
    for s in range(9):
        out[:, s * cout:(s + 1) * cout] = w[:, :, s // 3, s % 3].T
    return out


def _pad50(img):
    out = np.zeros((50, 50), np.float32)
    out[1:49, 1:49] = img
    return out


def _im2col9(img, rows, ones_row=False):
    """padded 50x50 -> [9(+1), rows*48] rows ordered s=dy*3+dx."""
    p = _pad50(img)
    rws = [p[dy:dy + rows, dx:dx + W].ravel()
           for dy in range(3) for dx in range(3)]
    if ones_row:
        rws.append(np.ones(rows * W, np.float32))
    return np.stack(rws)


def _blobs(inp, flip, c):
    w = {k: (inp[k][:, :, ::-1, :] if flip else inp[k])
         for k in ["enc1_w", "enc2_w", "bott_w", "dec2_w", "dec1_w",
                   "emb_w", "dsh_w"]}
    seg = {}
    seg["enc1s"] = w["enc1_w"].reshape(16, 3, 9).transpose(2, 1, 0) \
                              .reshape(27, 16)
    seg["enc2"] = _wT_flat(w["enc2_w"])
    seg["bott"] = _wT_flat(w["bott_w"])
    seg["dec2a"] = _wT_flat(w["dec2_w"][:, :64])
    seg["dec2b"] = _wT_flat(w["dec2_w"][:, 64:])
    seg["dec1a"] = _wT_flat(w["dec1_w"][:, :32])
    seg["dec1b"] = _wT_flat(w["dec1_w"][:, 32:])
    seg["out"] = inp["out_w"][:, :, 0, 0].T
    seg["outc"] = inp["out_w"][c, :, 0, 0][:, None]
    seg["emb"] = np.vstack([w["emb_w"].reshape(100, 9).T,
                            inp["emb_b"][None, :]])
    seg["dshc"] = w["dsh_w"].reshape(7, 9)

    def pack(segs, ncols):
        blob = np.zeros((64, ncols), np.float16)
        off = 0
        for nm, rows, cols in segs:
            blob[0:rows, off:off + cols] = seg[nm].astype(np.float16)
            off += cols
        return blob

    blobbias = np.zeros((64, 8), np.float32)
    for nm, col in BIAS_COL.items():
        if nm == "outc":
            v = inp["out_b"][c:c + 1]
        else:
            v = inp[nm + "_b"]
        blobbias[0:len(v), col] = v
    return pack(ASEGS, ACOLS), pack(BSEGS, BCOLS), blobbias


def make_in_maps(inp):
    maps = []
    for k8 in range(8):
        n_idx, half = k8 // 2, k8 % 2
        b, c = CORE_BC[n_idx]
        x1c, x2c, x3b = inp["x1"][b, c], inp["x2"][b, c], inp["x3"][b]
        if half:
            x1c, x2c, x3b = x1c[::-1], x2c[::-1], x3b[:, ::-1]
        # enc1 im2col [27, 39*48]: row s*3+ci
        bx1 = np.zeros((27, 39 * W), np.float32)
        for ci in range(3):
            im9 = _im2col9(x3b[ci], 39)
            for s in range(9):
                bx1[s * 3 + ci] = im9[s]
        bx2 = np.concatenate([_im2col9(x1c, H, True),
                              _im2col9(x2c, H, True)], axis=1)
        bx3 = x2c[0:25, :].reshape(1, Q)
        blobA, blobB, blobbias = _blobs(inp, bool(half), c)
        maps.append({"blobA": blobA, "blobB": blobB,
                     "blobBias": blobbias,
                     "blobX1": np.ascontiguousarray(bx1.astype(np.float16)),
                     "blobX2": np.ascontiguousarray(bx2.astype(np.float16)),
                     "blobX3": np.ascontiguousarray(bx3.astype(np.float16))})
    return maps


def assemble(results):
    out = np.zeros((2, 2, H, W), np.float32)
    for k8, r in enumerate(results):
        n_idx, half = k8 // 2, k8 % 2
        b, c = CORE_BC[n_idx]
        y = r["out"].reshape(24, W)
        if half == 0:
            out[b, c - 2, 0:24] = y
        else:
            out[b, c - 2, 24:48] = y[::-1]
    return out


def kernel(**inputs):
    inp = {k: np.asarray(v) for k, v in inputs.items()}
    nc = _get_program()
    maps = make_in_maps(inp)
    res = run_bass_kernel_spmd(nc, maps, core_ids=list(range(8)), trace=False)
    return assemble(res.results)


# revision 11
# speedup vs baseline: 1.1519x; 1.1173x over previous
"""FEELVOS fused kernel for TRN2, 8-core SPMD — fp16 rev2.

Sharding: the reference only returns logits for classes C-2, C-1, so only 4 of
the 8 fused (batch, class) items matter. 8 cores = 4 (b, c) pairs x 2 frame
halves (top/bottom 24 rows). Bottom-half cores receive row-flipped inputs and
row-flipped conv kernels so every core runs the identical program computing
"top 25 rows" of its (possibly flipped) frame; the host un-flips on gather.

All matmul operands fp16 (1 col/cycle on the PE vs ~3 for fp32-HIGH); PSUM
stays fp32. |a|^2 is carried as an fp16 hi+lo pair (rows 100/101, K=102).

U-Net decoder convs read single concatenated-K tiles: skip connections are
written at partition offsets via matmul tile_position (enc1 -> cat1[32:48],
enc2 -> cat2[64:96]), halving the decoder matmul count. GPSIMD cannot access
PSUM, so the distance-matrix min is drained by two engines: DVE reduces some
PSUM tiles directly; for the rest ACT copies PSUM->fp16 SBUF and DVE reduces
at double rate. The head conv runs directly on the padded [7,2500] xt plane
(9 shifted K=7 matmuls), no im2col.
"""
import numpy as np

import concourse.bass as bass
import concourse.bacc as bacc
import concourse.tile as tile
from concourse import mybir
from concourse.bass_utils import run_bass_kernel_spmd
from concourse.masks import make_identity

F32 = mybir.dt.float32
F16 = mybir.dt.float16
AF = mybir.ActivationFunctionType
ALU = mybir.AluOpType
AX = mybir.AxisListType

H = W = 48
NPIX = H * W                 # 2304 ref pixels
QROWS = 25
Q = QROWS * W                # 1200 query pixels
QCH, NQC = 120, 10           # query chunking for the distance matmul
ECH = [512, 512, 512, 512, 256]   # 2304 column chunking (PSUM bank)
E3CH = [432, 384, 384]            # 1200 column chunking
DIRECT_C = (0, 4, 8)         # chunks whose PSUM is reduced directly by DVE

_PROG = None


def _r3(ap, h, w):
    return ap.rearrange("c (h w) -> c h w", h=h, w=w)


# blob column layouts: (name, row0, nrows, cols), fp16. Row offsets place
# weights at the partition base their matmul's contraction rows need.
ASEGS = [("enc1s", 0, 27, 16), ("enc2", 32, 16, 288), ("bott", 64, 32, 576)]
BSEGS = [("dec2", 0, 96, 288), ("dec1", 0, 48, 144), ("out", 0, 16, 4),
         ("outc", 0, 16, 1), ("emb", 0, 10, 100), ("dshc", 0, 7, 9)]


def _offsets(segs):
    off, o = {}, 0
    for nm, _r0, _r, c in segs:
        off[nm] = o
        o += c
    return off, o


AOFF, ACOLS = _offsets(ASEGS)
BOFF, BCOLS = _offsets(BSEGS)
# blobBias [96, 10] fp32: cols 0-7 at row 0; col 8 = enc1_b at rows 32..47,
# col 9 = enc2_b at rows 64..95 (lane-aligned with offset ACTs).
BIAS_COL = {"enc1": 0, "enc2": 1, "bott": 2, "dec2": 3, "dec1": 4,
            "out": 5, "dsh": 6, "outc": 7}


def _emit(nc, tc, ctx):
    # ------------------------------------------------------------- dram io
    bA = nc.dram_tensor("blobA", [96, ACOLS], F16, kind="ExternalInput").ap()
    bB = nc.dram_tensor("blobB", [96, BCOLS], F16, kind="ExternalInput").ap()
    bBias = nc.dram_tensor("blobBias", [96, 10], F32,
                           kind="ExternalInput").ap()
    bX1 = nc.dram_tensor("blobX1", [27, 39 * W], F16,
                         kind="ExternalInput").ap()
    bX2 = nc.dram_tensor("blobX2", [10, 2 * NPIX], F16,
                         kind="ExternalInput").ap()
    bX3 = nc.dram_tensor("blobX3", [1, Q], F16, kind="ExternalInput").ap()
    out_d = nc.dram_tensor("out", [1, 24 * W], F32, kind="ExternalOutput").ap()

    # ------------------------------------------------------------- sbuf
    sb = ctx.enter_context(tc.tile_pool(name="sb", bufs=1))

    def st(name, p, f, dt=F16):
        return sb.tile([p, f], dt, tag=name, name=name)

    bloba = st("bloba", 96, ACOLS)
    blobb = st("blobb", 96, BCOLS)
    blobbias = st("blobbias", 96, 10, F32)
    im27 = st("im27", 27, 39 * W)        # enc1 im2col (host-built)
    im2c12 = st("im2c12", 10, 2 * NPIX)  # emb im2col e1|e2 (host-built)

    def wseg(blob, off, segs, nm):
        r0, rows, cols = next((a, b, c) for n, a, b, c in segs if n == nm)
        return blob[r0:r0 + rows, off[nm]:off[nm] + cols]

    wt = {nm: wseg(bloba, AOFF, ASEGS, nm) for nm, _, _, _ in ASEGS}
    wt.update({nm: wseg(blobb, BOFF, BSEGS, nm) for nm, _, _, _ in BSEGS})
    bia = {nm: blobbias[0:r, c:c + 1]
           for nm, (r, c) in {"enc1": (16, 0), "enc2": (32, 1),
                              "bott": (64, 2), "dec2": (32, 3),
                              "dec1": (16, 4), "out": (4, 5),
                              "dsh": (1, 6), "outc": (1, 7)}.items()}
    bia["enc1@32"] = blobbias[32:48, 8:9]
    bia["enc2@64"] = blobbias[64:96, 9:10]

    # device-written padded planes (fp16)
    x3cp = st("x3cp", 1, 2500)
    cat1 = st("cat1", 48, 2500)   # rows 0..31 up(d2), rows 32..47 e1
    p1p = st("p1p", 48, 676)      # rows 32..47 used
    cat2 = st("cat2", 96, 676)    # rows 0..63 up(bt), rows 64..95 e2
    p2p = st("p2p", 96, 196)      # rows 64..95 used
    btp = st("btp", 64, 196)
    d2p = st("d2p", 32, 676)
    d1p = st("d1p", 16, 2500)
    xt = st("xt", 7, 2500)

    im2c3 = st("im2c3", 10, Q)           # emb im2col (e3)
    e1x = st("e1x", 102, NPIX)           # rows 0..99 = -2*e1, 100/101 hi/lo
    e2x = st("e2x", 102, NPIX)
    e3x = st("e3x", 102, Q)              # rows 0..99 = e3, 100/101 = ones
    esq1 = st("esq1", 100, NPIX)
    esq2 = st("esq2", 100, NPIX)
    esq3 = st("esq3", 100, Q)
    ident = st("ident", 128, 128)

    c025 = st("c025", 100, 1)
    c1 = st("c1", 100, 2)
    hib1 = st("hib1", 1, NPIX)
    lob1 = st("lob1", 1, NPIX)
    hib2 = st("hib2", 1, NPIX)
    lob2 = st("lob2", 1, NPIX)
    b2all = st("b2all", QCH, NQC, F32)
    minsG = st("minsG", QCH, 2 * NQC, F32)
    minsL = st("minsL", QCH, 2 * NQC, F32)
    out_sb = st("out_sb", 1, 24 * W, F32)

    small = ctx.enter_context(tc.tile_pool(name="small", bufs=8))
    tmp = ctx.enter_context(tc.tile_pool(name="tmp", bufs=2))
    scr = ctx.enter_context(tc.tile_pool(name="scr", bufs=2))

    # ------------------------------------------------------------- input dma
    nc.sync.dma_start(bloba[:, 0:16], bA[:, 0:16])          # enc1s
    nc.sync.dma_start(blobbias[:], bBias)
    nc.sync.dma_start(im27[:, 0:960], bX1[:, 0:960])        # enc1 rows 0..19
    nc.sync.dma_start(bloba[:, 16:ACOLS], bA[:, 16:ACOLS])  # enc2+bott
    nc.sync.dma_start(im27[:, 960:39 * W], bX1[:, 960:39 * W])
    nc.sync.dma_start(blobb[:], bB)
    nc.sync.dma_start(im2c12[:, 0:NPIX], bX2[:, 0:NPIX])
    nc.sync.dma_start(im2c12[:, NPIX:2 * NPIX], bX2[:, NPIX:2 * NPIX])

    # ------------------------------------------------------------- init
    make_identity(nc, ident[:])
    nc.gpsimd.memset(c025[:], 0.25)
    nc.gpsimd.memset(c1[:], 1.0)
    nc.gpsimd.memset(xt[:], 0.0)
    # engine partition starts must be 32-aligned; rows 96..99 / 0..8 are
    # overwritten later by the embconv ACT / shift DMAs.
    nc.vector.memset(e3x[96:102, :], 1.0)
    nc.vector.memset(im2c3[0:10, :], 1.0)

    xt3 = _r3(xt[:], 50, 50)
    x3cp3 = _r3(x3cp[:], 50, 50)

    def borders(eng, ap3, pw):
        eng.memset(ap3[:, 0:1, :], 0.0)
        eng.memset(ap3[:, pw - 1:pw, :], 0.0)
        eng.memset(ap3[:, 1:pw - 1, 0:1], 0.0)
        eng.memset(ap3[:, 1:pw - 1, pw - 1:pw], 0.0)

    cat13 = _r3(cat1[:], 50, 50)
    p1p3 = _r3(p1p[:], 26, 26)
    cat23 = _r3(cat2[:], 26, 26)
    p2p3 = _r3(p2p[:], 14, 14)
    btp3 = _r3(btp[:], 14, 14)
    d2p3 = _r3(d2p[:], 26, 26)
    d1p3 = _r3(d1p[:], 50, 50)

    borders(nc.gpsimd, x3cp3, 50)
    borders(nc.gpsimd, cat13, 50)
    borders(nc.gpsimd, p1p3[32:48], 26)
    borders(nc.vector, cat23, 26)
    borders(nc.vector, p2p3[64:96], 14)
    borders(nc.vector, btp3, 14)
    borders(nc.vector, d2p3, 26)
    borders(nc.gpsimd, d1p3, 50)
    # xt ch6 = x2 rows 0..24 straight from dram (after the xt memset)
    nc.gpsimd.dma_start(xt3[6:7, 1:26, 1:49], bX3)

    pconv = ctx.enter_context(tc.tile_pool(name="pconv", bufs=2, space="PSUM"))
    pmain = ctx.enter_context(tc.tile_pool(name="pmain", bufs=2, space="PSUM"))

    # ------------------------------------------------------------ helpers
    def conv9(src3, wtile, cin, cout, row_chunks, w_, func, bias_ap, dst3,
              pbase=0, obase=0):
        """3x3 conv via 9 shifted matmuls accumulating in PSUM.
        pbase: partition base of src/weights; obase: of the PSUM output."""
        tp = (pbase, obase) if (pbase or obase) else None
        s3 = src3[pbase:pbase + cin]
        r0 = 0
        for nr in row_chunks:
            ps = pconv.tile([obase + cout, nr * w_], F32, tag="conv",
                            name="convps")
            for s in range(9):
                dy, dx = s // 3, s % 3
                nc.tensor.matmul(ps[obase:obase + cout, :],
                                 wtile[:, s * cout:(s + 1) * cout],
                                 s3[:, r0 + dy:r0 + dy + nr, dx:dx + w_],
                                 start=(s == 0), stop=(s == 8),
                                 tile_position=tp)
            nc.scalar.activation(dst3[obase:obase + cout,
                                      1 + r0:1 + r0 + nr, 1:1 + w_],
                                 _r3(ps[obase:obase + cout, :], nr, w_),
                                 func, bias=bias_ap)
            r0 += nr

    def pool2(src3, dst3, orows, ocols, pbase, cch):
        t1 = tmp.tile([pbase + cch, orows * ocols], F16, tag="pool_a",
                      name="poolt1")
        t2 = tmp.tile([pbase + cch, orows * ocols], F16, tag="pool_b",
                      name="poolt2")
        s3 = src3[pbase:pbase + cch]
        v = [s3[:, 1 + a:1 + a + 2 * orows:2, 1 + b:1 + b + 2 * ocols:2]
             for a, b in ((0, 0), (1, 1), (0, 1), (1, 0))]
        t13 = _r3(t1[pbase:pbase + cch, :], orows, ocols)
        t23 = _r3(t2[pbase:pbase + cch, :], orows, ocols)
        nc.vector.tensor_max(t13, v[0], v[1])
        nc.vector.tensor_max(t23, v[2], v[3])
        nc.vector.tensor_max(dst3[pbase:pbase + cch, 1:1 + orows,
                                  1:1 + ocols], t13, t23)

    def up2(src3, sbase, dst3, dbase, cch, irows, icols):
        s = src3[sbase:sbase + cch, 1:1 + irows, 1:1 + icols]
        for a in (0, 1):
            for b in (0, 1):
                nc.vector.tensor_copy(
                    dst3[dbase:dbase + cch, 1 + a:1 + a + 2 * irows:2,
                         1 + b:1 + b + 2 * icols:2], s)

    def embconv(imbuf, chunks, dst, scale):
        off = 0
        for cw in chunks:
            ps = pconv.tile([100, cw], F32, tag="conv", name="convps")
            nc.tensor.matmul(ps[:], wt["emb"], imbuf[:, off:off + cw],
                             start=True, stop=True)
            nc.scalar.activation(dst[0:100, off:off + cw], ps[:],
                                 AF.Copy, scale=scale)
            off += cw

    # ------------------------------------------------------------- U-Net
    # enc1: im2col matmuls -> cat1[32:48] (tile_position col offset 32)
    r0 = 0
    for nr in (10, 10, 10, 8):
        ps = pconv.tile([48, nr * W], F32, tag="conv", name="convps")
        nc.tensor.matmul(ps[32:48, :], wt["enc1s"],
                         im27[:, r0 * W:(r0 + nr) * W],
                         start=True, stop=True, tile_position=(0, 32))
        nc.scalar.activation(cat13[32:48, 1 + r0:1 + r0 + nr, 1:1 + W],
                             _r3(ps[32:48, :], nr, W), AF.Relu,
                             bias=bia["enc1@32"])
        r0 += nr
    pool2(cat13, p1p3, 19, 24, 32, 16)
    conv9(p1p3, wt["enc2"], 16, 32, [18], 24, AF.Relu, bia["enc2@64"],
          cat23, pbase=32, obase=64)
    pool2(cat23, p2p3, 9, 12, 64, 32)
    conv9(p2p3, wt["bott"], 32, 64, [8], 12, AF.Relu, bia["bott"], btp3,
          pbase=64, obase=0)
    up2(btp3, 0, cat23, 0, 64, 8, 12)
    conv9(cat23, wt["dec2"], 96, 32, [14], 24, AF.Relu, bia["dec2"], d2p3)
    up2(d2p3, 0, cat13, 0, 32, 14, 24)
    conv9(cat13, wt["dec1"], 48, 16, [10, 10, 6], W, AF.Relu, bia["dec1"],
          d1p3)

    # 1x1 output conv -> xt[0:4] (all 4 channels) and x3cp (class-c channel)
    r0 = 0
    for nr in (10, 10, 6):
        rhs = d1p3[:, 1 + r0:1 + r0 + nr, 1:1 + W]
        ps = pconv.tile([4, nr * W], F32, tag="conv", name="convps")
        nc.tensor.matmul(ps[:], wt["out"], rhs, start=True, stop=True)
        nc.scalar.activation(xt3[0:4, 1 + r0:1 + r0 + nr, 1:1 + W],
                             _r3(ps[:], nr, W), AF.Identity, bias=bia["out"])
        psc = pconv.tile([1, nr * W], F32, tag="conv", name="convps")
        nc.tensor.matmul(psc[:], wt["outc"], rhs, start=True, stop=True)
        nc.scalar.activation(x3cp3[0:1, 1 + r0:1 + r0 + nr, 1:1 + W],
                             _r3(psc[:], nr, W), AF.Identity,
                             bias=bia["outc"])
        r0 += nr

    # ------------------------------------------------- ref embeddings
    # PE: emb1 x5, emb2 x5, then per-chunk sq matmuls; ACT pipelines the
    # squares per chunk so the PE never waits on one big Square.
    embconv(im2c12[0:10, 0:NPIX], ECH, e1x[:], -2.0)
    embconv(im2c12[0:10, NPIX:2 * NPIX], ECH, e2x[:], -2.0)

    def sqrow(ex, esq, hib, lob):
        off = 0
        for cw in ECH:
            nc.scalar.activation(esq[:, off:off + cw], ex[0:100, off:off + cw],
                                 AF.Square)
            ps = pconv.tile([1, cw], F32, tag="conv", name="sqps")
            nc.tensor.matmul(ps[:], c025[:], esq[:, off:off + cw],
                             start=True, stop=True)
            nc.scalar.copy(hib[0:1, off:off + cw], ps[:])
            nc.vector.scalar_tensor_tensor(lob[0:1, off:off + cw], ps[:], 1.0,
                                           hib[0:1, off:off + cw],
                                           op0=ALU.mult, op1=ALU.subtract)
            off += cw
        nc.sync.dma_start(ex[100:101, 0:NPIX], hib[0:1, 0:NPIX])
        nc.sync.dma_start(ex[101:102, 0:NPIX], lob[0:1, 0:NPIX])

    sqrow(e1x[:], esq1[:], hib1, lob1)
    sqrow(e2x[:], esq2[:], hib2, lob2)

    # ------------------------------------------------------- embedding 3
    for s in range(9):
        dy, dx = s // 3, s % 3
        eng = nc.sync if s % 2 == 0 else nc.gpsimd
        eng.dma_start(im2c3[s:s + 1, 0:Q],
                      x3cp3[0:1, dy:dy + QROWS, dx:dx + W])
    embconv(im2c3[:], E3CH, e3x[:], 1.0)
    nc.scalar.activation(esq3[:, 0:Q], e3x[0:100, 0:Q], AF.Square)

    # |b|^2 columns [120, 1] per chunk -> b2all
    for c in range(NQC):
        b2ps = pconv.tile([QCH, 2], F32, tag="conv", name="b2ps")
        nc.tensor.matmul(b2ps[:], esq3[:, c * QCH:(c + 1) * QCH], c1[:],
                         start=True, stop=True)
        nc.scalar.copy(b2all[:, c:c + 1], b2ps[:, 0:1])

    # ------------------------------------------------------- matching
    def match_chunks(ex, minsR, c_range):
        for c in c_range:
            lhsT = e3x[:, c * QCH:(c + 1) * QCH]
            direct = c in DIRECT_C
            for j in range(2):
                ps = pmain.tile([QCH, 1152], F32, tag="main", name="mainps")
                base = j * 1152
                for o, n in ((0, 512), (512, 512), (1024, 128)):
                    nc.tensor.matmul(ps[:, o:o + n], lhsT,
                                     ex[:][:, base + o:base + o + n],
                                     start=True, stop=True)
                col = minsR[:, 2 * c + j:2 * c + j + 1]
                if direct:
                    nc.vector.tensor_reduce(col, ps[:], axis=AX.X, op=ALU.min)
                else:
                    sc = scr.tile([QCH, 1152], F16, tag="scr", name="scrt")
                    nc.scalar.copy(sc[:], ps[:])
                    nc.vector.tensor_reduce(col, sc[:], axis=AX.X, op=ALU.min)

    def ref_tail(minsR):
        dmin10 = small.tile([QCH, NQC], F32, tag="dmin", name="dmin")
        nc.vector.tensor_reduce(
            dmin10[:], minsR[:].rearrange("p (c two) -> p c two", two=2),
            axis=AX.X, op=ALU.min)
        dsum = small.tile([QCH, NQC], F32, tag="dsum", name="dsum")
        nc.vector.scalar_tensor_tensor(dsum[:], dmin10[:], 1.0, b2all[:],
                                       op0=ALU.mult, op1=ALU.add)
        dmax = small.tile([QCH, NQC], F32, tag="dmax", name="dmax")
        nc.vector.tensor_scalar(dmax[:], dsum[:], 0.0, 0.0,
                                op0=ALU.max, op1=ALU.max)
        gcol = small.tile([QCH, NQC], F16, tag="gcol", name="gcol")
        nc.scalar.activation(gcol[:], dmax[:], AF.Tanh, scale=0.5)
        return gcol

    def ref_emit_plane(gcol, r, eng):
        pst = pconv.tile([NQC, QCH], F16, tag="conv", name="gmtps")
        nc.tensor.transpose(pst[:], gcol[:], ident[:QCH, :QCH])
        gcolT = small.tile([NQC, QCH], F16, tag="gcolT", name="gcolT")
        nc.scalar.copy(gcolT[:], pst[:])
        gflat = small.tile([1, Q], F16, tag="gflat", name="gflat")
        eng.dma_start(gflat[:], gcolT[:])
        eng.dma_start(xt3[4 + r:5 + r, 1:26, 1:49], gflat[:])

    match_chunks(e1x, minsG, range(NQC))          # gm matmuls + reduces
    gcol_g = ref_tail(minsG)                      # overlaps lm matmuls below
    match_chunks(e2x, minsL, range(3))
    ref_emit_plane(gcol_g, 0, nc.gpsimd)          # PE transpose: no stall here
    match_chunks(e2x, minsL, range(3, NQC))
    gcol_l = ref_tail(minsL)
    ref_emit_plane(gcol_l, 1, nc.sync)

    # ------------------------------------------------------- head conv
    r0 = 0
    for nr in (8, 8, 8):
        ps = pconv.tile([1, nr * W], F32, tag="conv", name="convps")
        for s in range(9):
            dy, dx = s // 3, s % 3
            nc.tensor.matmul(ps[:], wt["dshc"][:, s:s + 1],
                             xt3[:, r0 + dy:r0 + dy + nr, dx:dx + W],
                             start=(s == 0), stop=(s == 8))
        nc.scalar.activation(out_sb[0:1, r0 * W:(r0 + nr) * W],
                             _r3(ps[:], nr, W), AF.Identity, bias=bia["dsh"])
        r0 += nr
    nc.sync.dma_start(out_d, out_sb[:])


def build_program():
    import contextlib
    nc = bacc.Bacc("TRN2", target_bir_lowering=False, debug=False,
                   num_devices=8)
    with tile.TileContext(nc) as tc:
        with contextlib.ExitStack() as ctx:
            _emit(nc, tc, ctx)
    nc.compile()
    return nc


def _get_program():
    global _PROG
    if _PROG is None:
        _PROG = build_program()
    return _PROG


CORE_BC = [(0, 2), (0, 3), (1, 2), (1, 3)]


def _wT_flat(w):
    """[Cout, Cin, 3, 3] -> [Cin, 9*Cout]: col block s holds w[:, :, s//3, s%3].T"""
    cout, cin = w.shape[:2]
    out = np.zeros((cin, 9 * cout), np.float32)
    for s in range(9):
        out[:, s * cout:(s + 1) * cout] = w[:, :, s // 3, s % 3].T
    return out


def _pad50(img):
    out = np.zeros((50, 50), np.float32)
    out[1:49, 1:49] = img
    return out


def _im2col9(img, rows, ones_row=False):
    """padded 50x50 -> [9(+1), rows*48] rows ordered s=dy*3+dx."""
    p = _pad50(img)
    rws = [p[dy:dy + rows, dx:dx + W].ravel()
           for dy in range(3) for dx in range(3)]
    if ones_row:
        rws.append(np.ones(rows * W, np.float32))
    return np.stack(rws)


def _blobs(inp, flip, c):
    w = {k: (inp[k][:, :, ::-1, :] if flip else inp[k])
         for k in ["enc1_w", "enc2_w", "bott_w", "dec2_w", "dec1_w",
                   "emb_w", "dsh_w"]}
    seg = {}
    seg["enc1s"] = w["enc1_w"].reshape(16, 3, 9).transpose(2, 1, 0) \
                              .reshape(27, 16)
    seg["enc2"] = _wT_flat(w["enc2_w"])
    seg["bott"] = _wT_flat(w["bott_w"])
    seg["dec2"] = _wT_flat(w["dec2_w"])
    seg["dec1"] = _wT_flat(w["dec1_w"])
    seg["out"] = inp["out_w"][:, :, 0, 0].T
    seg["outc"] = inp["out_w"][c, :, 0, 0][:, None]
    seg["emb"] = np.vstack([w["emb_w"].reshape(100, 9).T,
                            inp["emb_b"][None, :]])
    seg["dshc"] = w["dsh_w"].reshape(7, 9)

    def pack(segs, ncols):
        blob = np.zeros((96, ncols), np.float16)
        off = 0
        for nm, r0, rows, cols in segs:
            blob[r0:r0 + rows, off:off + cols] = seg[nm].astype(np.float16)
            off += cols
        return blob

    blobbias = np.zeros((96, 10), np.float32)
    for nm, col in BIAS_COL.items():
        if nm == "outc":
            v = inp["out_b"][c:c + 1]
        else:
            v = inp[nm + "_b"]
        blobbias[0:len(v), col] = v
    blobbias[32:48, 8] = inp["enc1_b"]
    blobbias[64:96, 9] = inp["enc2_b"]
    return pack(ASEGS, ACOLS), pack(BSEGS, BCOLS), blobbias


def make_in_maps(inp):
    maps = []
    for k8 in range(8):
        n_idx, half = k8 // 2, k8 % 2
        b, c = CORE_BC[n_idx]
        x1c, x2c, x3b = inp["x1"][b, c], inp["x2"][b, c], inp["x3"][b]
        if half:
            x1c, x2c, x3b = x1c[::-1], x2c[::-1], x3b[:, ::-1]
        bx1 = np.zeros((27, 39 * W), np.float32)
        for ci in range(3):
            im9 = _im2col9(x3b[ci], 39)
            for s in range(9):
                bx1[s * 3 + ci] = im9[s]
        bx2 = np.concatenate([_im2col9(x1c, H, True),
                              _im2col9(x2c, H, True)], axis=1)
        bx3 = x2c[0:25, :].reshape(1, Q)
        blobA, blobB, blobbias = _blobs(inp, bool(half), c)
        maps.append({"blobA": blobA, "blobB": blobB,
                     "blobBias": blobbias,
                     "blobX1": np.ascontiguousarray(bx1.astype(np.float16)),
                     "blobX2": np.ascontiguousarray(bx2.astype(np.float16)),
                     "blobX3": np.ascontiguousarray(bx3.astype(np.float16))})
    return maps


def assemble(results):
    out = np.zeros((2, 2, H, W), np.float32)
    for k8, r in enumerate(results):
        n_idx, half = k8 // 2, k8 % 2
        b, c = CORE_BC[n_idx]
        y = r["out"].reshape(24, W)
        if half == 0:
            out[b, c - 2, 0:24] = y
        else:
            out[b, c - 2, 24:48] = y[::-1]
    return out


def kernel(**inputs):
    inp = {k: np.asarray(v) for k, v in inputs.items()}
    nc = _get_program()
    maps = make_in_maps(inp)
    res = run_bass_kernel_spmd(nc, maps, core_ids=list(range(8)), trace=False)
    return assemble(res.results)
